# revision 1
# baseline (speedup 1.0000x reference)
"""Trainium2 Bass kernel for nn_MemorizingTransformer (retrieval_knn).

Memorizing-transformer attention block: cosine-sim causal local attention with
per-query retrieved KNN memories, joint softmax over [memory | local], and
input/output projections.

Sharding: (b, h) across 8 cores — core c handles batch b=c//4 and heads
h0=2*(c%4), h0+1. Every core runs an identical NEFF (pure SPMD); only input
slices differ. The output projection is computed per-core on the core's two
head rows of w_out, giving partial sums that the host reduces.

Device algorithm per core (f32, optional f32r matmuls):
  phase X : xT = transpose(x[b])                      (PE transposes)
  phase KV: k_nat, v = x@w_k, x@w_v; sumsq(k)         (PE, ACT)
  phase Q : q_nat = x@w_q_h (both heads); sumsq(q)    (PE, ACT)
  norm    : one batched sqrt + reciprocal; scale k,q; kT/qT via PE transpose
  per head p:
    mem scores per q-block g:
      S_mem = sum_d(mem_k * q_s)                      (DVE mul + seg reduce)
      P_mem, rowsum = exp(scale*S_mem - scale)        (ACT fused accum)
    local, jt-outer with 4-block column batching:
      S_T[128j, <=512q] = kT.T @ qT_all               (PE)
      P_T = exp(scale*S_T - scale), tril on diagonal  (ACT, DVE)
      PV: psum_o[g] += P_T_g.T @ [v|1]                (PE accum) [128q,4,65]
    mem values per g (PE, block-diagonal trick):
      mv_t[(ql j), g4, d] staged so 4 stride-32 queries stack on partitions;
      stage/stageT hold P_mem block-diagonally; 32 small matmuls give
      pm[65, 128q] = [mem_v|1].T @ P_mem per query; transposed-accumulated
      into psum_o so col 64 = total softmax denominator.
    combine: oh = psum_o[:, :64] * recip(psum_o[:, 64]); hoT = transpose(oh)
  partial_out[g] = hoT_g.T @ w_out[2 head rows]       (PE)

Softmax needs no max-subtraction: scores are cosine sims in [-1,1] times
scale=exp(scale_param), so exp(scale*(s-1)) is bounded in (0, 1].
"""

import os
import numpy as np

HEADS = 8
D = 64
KNN = 32
B = 2
N = 2048
DIM = 512
P = 128
NB = N // P          # 16 query/key blocks
NCO = DIM // P       # 4 contraction chunks of the model dim
NCORES = 8
FP32R = bool(int(os.environ.get("BASS_FP32R", "1")))
MEMBF16 = bool(int(os.environ.get("BASS_MEMBF16", "0")))
_SKIP_LOCAL = bool(int(os.environ.get("SKIP_LOCAL", "0")))
_SKIP_MEMK = bool(int(os.environ.get("SKIP_MEMK", "0")))
_SKIP_MEMV = bool(int(os.environ.get("SKIP_MEMV", "0")))
PHASE_MARKS = []
_MSTATE = {}


def _mark(nc, name):
    cur = nc.next_id()
    if _MSTATE.get("name") is not None:
        PHASE_MARKS.append((_MSTATE["name"], _MSTATE["id"], cur))
    _MSTATE["name"] = name
    _MSTATE["id"] = cur


def _build(use_mbias: bool):
    import concourse.bass as bass
    import concourse.mybir as mybir
    import concourse.tile as tile
    from concourse import bacc

    f32 = mybir.dt.float32
    f32r = mybir.dt.float32r
    bf16 = mybir.dt.bfloat16
    mdt = bf16 if MEMBF16 else f32
    AX = mybir.AxisListType
    ACTF = mybir.ActivationFunctionType

    def rcast(ap):
        return ap.bitcast(f32r) if FP32R else ap

    nc = bacc.Bacc(None, target_bir_lowering=False, name="memxformer")
    PHASE_MARKS.clear()
    _MSTATE.clear()

    # ---- I/O ------------------------------------------------------------
    xb = nc.dram_tensor("xb", (N, DIM), f32, kind="ExternalInput")
    wq2 = nc.dram_tensor("wq2", (DIM, 2 * D), f32, kind="ExternalInput")
    wkv = nc.dram_tensor("wkv", (DIM, 2 * D), f32, kind="ExternalInput")
    wout2 = nc.dram_tensor("wout2", (2 * D, DIM), f32, kind="ExternalInput")
    # scales[:, 0:2] = exp(scale_param[h0 + p]); scales[:, 2:4] = -that
    scales = nc.dram_tensor("scales", (P, 4), f32, kind="ExternalInput")
    mk = nc.dram_tensor("mk", (2, NB, P, KNN, D), f32, kind="ExternalInput")
    mv = nc.dram_tensor("mv", (2, NB, P, KNN, D + 1), mdt, kind="ExternalInput")
    if use_mbias:
        mbias = nc.dram_tensor("mbias", (2, NB, P, KNN), f32, kind="ExternalInput")
    out = nc.dram_tensor("out", (N, DIM), f32, kind="ExternalOutput")

    # constants baked into the NEFF
    eye_np = np.eye(P, dtype=np.float32)
    tril_np = np.triu(np.ones((P, P), dtype=np.float32))  # keep j <= q
    eye_d = nc.inline_tensor(eye_np, name="eye_c")
    import ml_dtypes
    eye16_d = nc.inline_tensor(eye_np.astype(ml_dtypes.bfloat16), name="eye16_c")
    tril_d = nc.inline_tensor(tril_np, name="tril_c")

    with tile.TileContext(nc) as tc:
        with (
            tc.tile_pool(name="singles", bufs=1) as singles,
            tc.tile_pool(name="xin", bufs=4) as xin,
            tc.tile_pool(name="mem", bufs=3) as memp,
            tc.tile_pool(name="mvp", bufs=3) as mvp,
            tc.tile_pool(name="prods", bufs=2) as prods,
            tc.tile_pool(name="small", bufs=6) as small,
            tc.tile_pool(name="pt", bufs=3) as ptp,
            tc.tile_pool(name="stts", bufs=3) as stts,
            tc.tile_pool(name="pms", bufs=3) as pms,
            tc.tile_pool(name="outp", bufs=3) as outp,
            tc.tile_pool(name="ppt", bufs=2, space="PSUM") as ppt,
            tc.tile_pool(name="pp512", bufs=2, space="PSUM") as pp512,
            tc.tile_pool(name="ppo", bufs=4, space="PSUM") as ppo,
        ):
            # ---- constants / weights ------------------------------------
            eye_sb = singles.tile([P, P], f32, tag="eye")
            nc.sync.dma_start(eye_sb, eye_d[:, :])
            eye16_sb = singles.tile([P, P], bf16, tag="eye16")
            nc.sync.dma_start(eye16_sb, eye16_d[:, :])
            tril_sb = singles.tile([P, P], f32, tag="tril")
            nc.sync.dma_start(tril_sb, tril_d[:, :])
            sc_sb = singles.tile([P, 4], f32, tag="scales")
            nc.sync.dma_start(sc_sb, scales[:, :])
            wq_sb = singles.tile([P, NCO, 2 * D], f32, tag="wq")
            nc.sync.dma_start(wq_sb, wq2[:, :].rearrange("(co p) c -> p co c", p=P))
            wkv_sb = singles.tile([P, NCO, 2 * D], f32, tag="wkv")
            nc.sync.dma_start(wkv_sb, wkv[:, :].rearrange("(co p) c -> p co c", p=P))
            wout_st = singles.tile([P, DIM], f32, tag="wout_st")
            nc.sync.dma_start(wout_st, wout2[:, :])
            wout_sb = singles.tile([P, DIM], f32r if FP32R else f32, tag="wout")
            nc.scalar.copy(out=wout_sb, in_=wout_st)

            _mark(nc, "setup")
            # ---- x transpose: xT[p, co, n] = x[n, co*128 + p] ------------
            xT = singles.tile([P, NCO, N], f32, tag="xT")
            for nb in range(NB):
                x_t = xin.tile([P, DIM], f32, tag="xtile")
                nc.sync.dma_start(x_t, xb[nb * P:(nb + 1) * P, :])
                pt_ps = ppt.tile([P, NCO, P], f32, tag="tps")
                for co in range(NCO):
                    nc.tensor.transpose(pt_ps[:, co, :],
                                        x_t[:, co * P:(co + 1) * P], eye_sb)
                nc.scalar.copy(out=xT[:, :, nb * P:(nb + 1) * P], in_=pt_ps)

            _mark(nc, "xT")
            # ---- k/v/q natural projections + sumsq ----------------------
            k_all = singles.tile([P, NB, D], f32, tag="k_all")
            v_aug = singles.tile([P, NB, D + 1], f32, tag="vaug")
            nc.gpsimd.memset(v_aug[:, :, D:D + 1], 1.0)
            q_all = singles.tile([P, 2 * NB, D], f32, tag="q_all")
            # ss_all: cols 0:16 = k blocks, 16:48 = q blocks (2 heads x 16)
            ss_all = singles.tile([P, NB + 2 * NB], f32, tag="ss")
            junk = singles.tile([P, D], f32, tag="junk")

            for g in range(NB):
                qsl = slice(g * P, (g + 1) * P)
                qnat = ppt.tile([P, 2 * D], f32, tag="tps")
                for co in range(NCO):
                    nc.tensor.matmul(qnat, xT[:, co, qsl], wq_sb[:, co, :],
                                     start=(co == 0), stop=(co == NCO - 1))
                for p in range(2):
                    idx = p * NB + g
                    nc.scalar.copy(out=q_all[:, idx, :],
                                   in_=qnat[:, p * D:(p + 1) * D])
                    nc.scalar.activation(out=junk, in_=qnat[:, p * D:(p + 1) * D],
                                         func=ACTF.Square,
                                         accum_out=ss_all[:, NB + idx:NB + idx + 1])

            _mark(nc, "kvqnat")
            # q norm scale first (memk can start as soon as q_s is ready)
            nrm_q = singles.tile([P, 2 * NB], f32, tag="nrm_q")
            nc.scalar.sqrt(nrm_q, ss_all[:, NB:3 * NB])
            rn_q = singles.tile([P, 2 * NB], f32, tag="rn_q")
            nc.vector.reciprocal(rn_q, nrm_q)
            q_s = singles.tile([P, 2 * NB, D], f32, tag="q_s")
            qT_all = singles.tile([D, 2 * NB, P], f32r if FP32R else f32, tag="qT")
            for idxc in range(0, 2 * NB, 4):
                qt_ps = ppt.tile([D, 4, P], f32, tag="tps")
                for i4 in range(4):
                    idx = idxc + i4
                    nc.vector.tensor_scalar_mul(q_s[:, idx, :], q_all[:, idx, :],
                                                rn_q[:, idx:idx + 1])
                    nc.tensor.transpose(qt_ps[:, i4, :], q_s[:, idx, :], eye_sb)
                nc.scalar.copy(out=qT_all[:, idxc:idxc + 4, :], in_=qt_ps)

            for jt in range(NB):
                ksl = slice(jt * P, (jt + 1) * P)
                kvnat = ppt.tile([P, 2 * D], f32, tag="tps")
                for co in range(NCO):
                    nc.tensor.matmul(kvnat, xT[:, co, ksl], wkv_sb[:, co, :],
                                     start=(co == 0), stop=(co == NCO - 1))
                nc.scalar.copy(out=k_all[:, jt, :], in_=kvnat[:, 0:D])
                nc.scalar.activation(out=junk, in_=kvnat[:, 0:D], func=ACTF.Square,
                                     accum_out=ss_all[:, jt:jt + 1])
                nc.scalar.copy(out=v_aug[:, jt, 0:D], in_=kvnat[:, D:2 * D])

            nrm_k = singles.tile([P, NB], f32, tag="nrm_k")
            nc.scalar.sqrt(nrm_k, ss_all[:, 0:NB])
            rn_k = singles.tile([P, NB], f32, tag="rn_k")
            nc.vector.reciprocal(rn_k, nrm_k)
            kT = singles.tile([D, NB, P], f32r if FP32R else f32, tag="kT")
            for jtc in range(0, NB, 4):
                kt_ps = ppt.tile([D, 4, P], f32, tag="tps")
                for j4 in range(4):
                    jt = jtc + j4
                    ktmp = small.tile([P, D], f32, tag="ktmp")
                    nc.vector.tensor_scalar_mul(ktmp, k_all[:, jt, :],
                                                rn_k[:, jt:jt + 1])
                    nc.tensor.transpose(kt_ps[:, j4, :], ktmp, eye_sb)
                nc.scalar.copy(out=kT[:, jtc:jtc + 4, :], in_=kt_ps)

            _mark(nc, "norm")
            # ---- head-output accumulator --------------------------------
            hoT = singles.tile([P, NB, P], f32r if FP32R else f32, tag="hoT")
            # staging for block-diagonal P_mem (manual double buffer; the
            # off-diagonal zeros are written once and never touched again)
            st2 = singles.tile([P, 2, 4, P], mdt, tag="st2")
            nc.gpsimd.memset(st2, 0.0)

            for p in range(2):
                sc_ap = sc_sb[:, p:p + 1]
                nb_ap = sc_sb[:, 2 + p:3 + p]

                _mark(nc, f"memk")
                # --- memory attention scores for all 16 blocks ---
                p_mem_all = singles.tile([P, NB, KNN], mdt, tag=f"pmem{p}")
                for g in range(NB if not _SKIP_MEMK else 0):
                    idx = p * NB + g
                    mk_t = memp.tile([P, KNN, D], f32, tag="mk")
                    nc.sync.dma_start(mk_t, mk[p, g])
                    prod = prods.tile([P, KNN, D], f32, tag="prod")
                    nc.vector.tensor_mul(
                        prod, mk_t, q_s[:, idx, None, :].to_broadcast((P, KNN, D)))
                    s_mem = small.tile([P, KNN], f32, tag="smem")
                    nc.vector.reduce_sum(s_mem, prod, axis=AX.X)
                    if use_mbias:
                        mb_t = small.tile([P, KNN], f32, tag="mbias")
                        nc.sync.dma_start(mb_t, mbias[p, g])
                        nc.vector.tensor_add(s_mem, s_mem, mb_t)
                    nc.scalar.activation(out=p_mem_all[:, g, :], in_=s_mem,
                                         func=ACTF.Exp, bias=nb_ap, scale=sc_ap)

                _mark(nc, f"local")
                # --- local causal attention, jt-outer, 4-block columns ---
                psum_o = [ppo.tile([P, 4, D + 1], f32, tag="po", name=f"po{i}")
                           for i in range(4)]
                def local_tile(qc, jt):
                    g_lo = max(jt, 4 * qc)
                    g_hi = 4 * qc + 4
                    ng = g_hi - g_lo
                    i_lo = p * NB + g_lo
                    st_ps = pp512.tile([P, 512], f32, tag="st", name="st_ps")
                    nc.tensor.matmul(
                        st_ps[:, :ng * P], kT[:, jt, :],
                        qT_all[:, i_lo:i_lo + ng, :],
                        start=True, stop=True)
                    p_t = ptp.tile([P, 4, P], f32, tag="pt", name="p_t")
                    nc.scalar.activation(
                        out=p_t[:, :ng, :],
                        in_=st_ps[:, :ng * P].rearrange("p (g q) -> p g q", q=P),
                        func=ACTF.Exp, bias=nb_ap, scale=sc_ap)
                    if g_lo <= jt < g_hi:
                        di = jt - g_lo
                        nc.vector.tensor_mul(p_t[:, di, :], p_t[:, di, :],
                                             tril_sb)
                    for gi in range(ng):
                        g = g_lo + gi
                        nc.tensor.matmul(
                            psum_o[qc][:, g - 4 * qc, :], p_t[:, gi, :],
                            v_aug[:, jt, :],
                            start=(jt == 0 and gi == 0), stop=False)

                _mark(nc, f"memv")
                # qc-outer: each bank's local attention completes, then its
                # memory-value chain fires immediately (overlaps later banks)
                for qc4 in range(4):
                    if not _SKIP_LOCAL:
                        for jt in range(4 * qc4 + 4):
                            local_tile(qc4, jt)
                    gc = 4 * qc4
                    if _SKIP_MEMK:
                        oh_ps0 = ppt.tile([D, 4, P], f32, tag="tps")
                        for gi in range(4):
                            g = gc + gi
                            qc, gq = g // 4, g % 4
                            rcp = small.tile([P, 1], f32, tag="rcp")
                            nc.vector.reciprocal(rcp, psum_o[qc][:, gq, D:D + 1])
                            oh = small.tile([P, D], f32, tag="oh")
                            nc.vector.tensor_scalar_mul(
                                oh, psum_o[qc][:, gq, 0:D], rcp)
                            nc.tensor.transpose(oh_ps0[:, gi, :], oh, eye_sb)
                        nc.scalar.copy(out=hoT[p * D:(p + 1) * D, gc:gc + 4, :],
                                       in_=oh_ps0)
                        continue
                    stage4 = st2[:, (gc // 4) % 2, :, :]
                    for gi in range(4):
                        g = gc + gi
                        for k4 in range(4):
                            nc.gpsimd.tensor_copy(
                                out=stage4[32 * k4:32 * (k4 + 1), gi,
                                           32 * k4:32 * (k4 + 1)],
                                in_=p_mem_all[32 * k4:32 * (k4 + 1), g, :])
                    stt_ps = ppt.tile([P, 4, P], mdt, tag="tps")
                    for gi in range(4):
                        nc.tensor.transpose(stt_ps[:, gi, :], stage4[:, gi, :],
                                            eye16_sb if MEMBF16 else eye_sb)
                    stT = stts.tile([P, 4, P], mdt, tag="stT")
                    nc.scalar.copy(out=stT, in_=stt_ps)
                    pm_ps = pp512.tile([D + 1, 4, P], f32, tag="st")
                    for gi in range(4):
                        g = gc + gi
                        mv_t = mvp.tile([P, KNN, D + 1], mdt, tag="mv")
                        nc.sync.dma_start(mv_t, mv[p, g])
                        stT_v = stT[:, gi, :].rearrange("p (ql gf) -> p gf ql",
                                                        gf=KNN)
                        pm_v = pm_ps[:, gi, :].rearrange("p (ql gf) -> p gf ql",
                                                         gf=KNN)
                        for g4 in range(KNN):
                            nc.tensor.matmul(pm_v[:, g4, :], mv_t[:, g4, :],
                                             stT_v[:, g4, :],
                                             start=True, stop=True)
                    pm_sb = pms.tile([D + 1, 4, P], f32, tag="pm")
                    nc.scalar.copy(out=pm_sb, in_=pm_ps)
                    oh_ps = ppt.tile([D, 4, P], f32, tag="tps")
                    for gi in range(4):
                        g = gc + gi
                        qc, gq = g // 4, g % 4
                        nc.tensor.matmul(psum_o[qc][:, gq, :], pm_sb[:, gi, :],
                                         eye_sb[0:D + 1, 0:D + 1],
                                         is_transpose=True, start=_SKIP_LOCAL,
                                         stop=True)
                        rcp = small.tile([P, 1], f32, tag="rcp")
                        nc.vector.reciprocal(rcp, psum_o[qc][:, gq, D:D + 1])
                        oh = small.tile([P, D], f32, tag="oh")
                        nc.vector.tensor_scalar_mul(oh, psum_o[qc][:, gq, 0:D],
                                                    rcp)
                        nc.tensor.transpose(oh_ps[:, gi, :], oh, eye_sb)
                    nc.scalar.copy(out=hoT[p * D:(p + 1) * D, gc:gc + 4, :],
                                   in_=oh_ps)

            _mark(nc, "outproj")
            # ---- output projection (partial: this core's two head rows) --
            for g in range(NB):
                pf = pp512.tile([P, DIM], f32, tag="st")
                nc.tensor.matmul(pf, hoT[:, g, :], wout_sb,
                                 start=True, stop=True)
                of_s = outp.tile([P, DIM], f32, tag="ofs")
                nc.scalar.copy(out=of_s, in_=pf)
                nc.sync.dma_start(out[g * P:(g + 1) * P, :], of_s)

    _mark(nc, "tile_finish")
    nc.compile()
    _mark(nc, None)
    return nc


def _to_bf16(a):
    import ml_dtypes
    return np.ascontiguousarray(a.astype(ml_dtypes.bfloat16))


def _prep_mv(mv_slice):
    """[2,2048,32,64] -> [2,16,128,32,65] bf16: partition (ql j) stacks the 4
    stride-32 queries of each group; col 64 = 1.0 (softmax-denominator row)."""
    import ml_dtypes
    dt = ml_dtypes.bfloat16 if MEMBF16 else np.float32
    r = mv_slice.reshape(2, NB, 4, KNN, KNN, D).transpose(0, 1, 2, 4, 3, 5)
    out = np.empty((2, NB, P, KNN, D + 1), dtype=dt)
    out[..., :D] = r.reshape(2, NB, P, KNN, D).astype(dt)
    out[..., D] = 1.0
    return out


def _prepare_in_maps(x, w_q, w_kv, w_out, scale_param, mem_k, mem_v, mem_mask,
                     use_mbias):
    f = np.float32
    scales8 = np.exp(scale_param.reshape(HEADS).astype(f))
    in_maps = []
    for c in range(NCORES):
        b = c // 4
        h0 = 2 * (c % 4)
        sc = np.empty((P, 4), dtype=f)
        sc[:, 0] = scales8[h0]
        sc[:, 1] = scales8[h0 + 1]
        sc[:, 2] = -scales8[h0]
        sc[:, 3] = -scales8[h0 + 1]
        m = {
            "xb": np.ascontiguousarray(x[b], dtype=f),
            "wq2": np.ascontiguousarray(w_q[:, h0 * D:(h0 + 2) * D], dtype=f),
            "wkv": np.ascontiguousarray(w_kv, dtype=f),
            "wout2": np.ascontiguousarray(w_out[h0 * D:(h0 + 2) * D, :], dtype=f),
            "scales": sc,
            "mk": np.ascontiguousarray(
                mem_k[b, h0:h0 + 2].reshape(2, NB, P, KNN, D), dtype=f),
            "mv": _prep_mv(mem_v[b, h0:h0 + 2]),
        }
        if use_mbias:
            mb = np.where(mem_mask[b, h0:h0 + 2], f(0), f(-1e30)).astype(f)
            m["mbias"] = np.ascontiguousarray(mb.reshape(2, NB, P, KNN))
        in_maps.append(m)
    return in_maps


def _run(x, w_q, w_kv, w_out, scale_param, mem_k, mem_v, mem_mask, trace=False):
    from concourse.bass_utils import run_bass_kernel_spmd

    use_mbias = not bool(np.all(mem_mask))
    nc = _build(use_mbias)
    in_maps = _prepare_in_maps(x, w_q, w_kv, w_out, scale_param,
                               mem_k, mem_v, mem_mask, use_mbias)
    res = run_bass_kernel_spmd(nc, in_maps, core_ids=list(range(NCORES)),
                               trace=trace)
    out = np.zeros((B, N, DIM), dtype=np.float32)
    for c in range(NCORES):
        out[c // 4] += res.results[c]["out"]
    return out, res


def kernel(x, w_q, w_kv, w_out, scale_param, mem_k, mem_v, mem_mask):
    trace = bool(int(os.environ.get("BASS_KERNEL_TRACE", "0")))
    out, _ = _run(x, w_q, w_kv, w_out, scale_param, mem_k, mem_v, mem_mask,
                  trace=trace)
    return out



# revision 15
# speedup vs baseline: 2.0963x; 2.0963x over previous
"""Trainium2 Bass kernel for nn_MemorizingTransformer (retrieval_knn).

Memorizing-transformer attention block: cosine-sim causal local attention with
per-query retrieved KNN memories, joint softmax over [memory | local], and
input/output projections.

Sharding: (b, h) across 8 cores — core c handles batch b=c//4 and heads
h0=2*(c%4), h0+1. Every core runs an identical NEFF (pure SPMD); only input
slices differ. The output projection is computed per-core on the core's two
head rows of w_out, giving partial sums that the host reduces.

v2 design (PE-pressure + DMA halving vs v1):
  - x arrives pre-transposed from host (xT); q/kv projections are fused into
    one matmul per (g, co) with f32r moving operand (full rate at N=256).
  - q-hat/k-hat are bf16; kT/qT via PE transpose (bf16, 1 cyc/row).
  - mem_k/mem_v shipped bf16 (halves HBM traffic).
  - local attention P*V uses the [65, q] orientation: stationary = v_aug
    (65th col = ones -> denominator row), moving = P_T tile. Accumulates in
    psum_oT[65, 4, 128] per (head, qc-chunk).
  - mem scores on DVE: bf16 mul + 2-stage bf16 tree-add + f32 reduce; exp on
    ACT; block-diagonal staging via DVE 32x32 stream-transpose + 4 copies.
  - mem value matmuls (stationary mem_v [128,65] bf16, moving stage [128,4])
    accumulate DIRECTLY into psum_oT (65th mv col = ones -> denominator).
  - denominator row -> [q, 1] via N=1 matmuls; outproj per head
    (stationary hoT chunk bf16, moving w_out rows bf16), divide folded into
    ACT scale-copy for head 0 and a gpsimd multiply-add for head 1.
"""

import os
import numpy as np

HEADS = 8
D = 64
KNN = 32
B = 2
N = 2048
DIM = 512
P = 128
NB = N // P          # 16 query/key blocks
NCO = DIM // P       # 4 contraction chunks of the model dim
NCORES = 8
# mem-score reduce: 2 = two bf16 tree stages + f32 reduce (default),
# 0 = plain reduce_sum, 6 = full bf16 tree
SCORE_TREE = int(os.environ.get("BASS_SCORE_TREE", "2"))


def _build(use_mbias: bool):
    import concourse.bass as bass
    import concourse.mybir as mybir
    import concourse.tile as tile
    from concourse import bacc
    import ml_dtypes

    f32 = mybir.dt.float32
    f32r = mybir.dt.float32r
    bf16 = mybir.dt.bfloat16
    AX = mybir.AxisListType
    ACTF = mybir.ActivationFunctionType
    MUL = mybir.AluOpType.mult
    ADD = mybir.AluOpType.add

    nc = bacc.Bacc(None, target_bir_lowering=False, name="memxformer")

    # ---- I/O ------------------------------------------------------------
    xbT = nc.dram_tensor("xbT", (NCO, P, N), f32, kind="ExternalInput")
    # fused [w_q (2 heads) | w_kv] chunks: [co, 128, 256]
    wqkv = nc.dram_tensor("wqkv", (NCO, P, 4 * D), f32, kind="ExternalInput")
    # per-head w_out rows, bf16: [2, 64, 512]
    wo = nc.dram_tensor("wo", (2, D, DIM), bf16, kind="ExternalInput")
    # scales[:, 0:2] = exp(scale_param[h0 + p]); scales[:, 2:4] = -that
    scales = nc.dram_tensor("scales", (P, 4), f32, kind="ExternalInput")
    mk = nc.dram_tensor("mk", (2, NB, P, KNN, D), bf16, kind="ExternalInput")
    mv = nc.dram_tensor("mv", (2, NB, P, KNN, D + 1), bf16, kind="ExternalInput")
    if use_mbias:
        mbias = nc.dram_tensor("mbias", (2, NB, P, KNN), f32, kind="ExternalInput")
    out = nc.dram_tensor("out", (N, DIM), f32, kind="ExternalOutput")

    # constants baked into the NEFF
    eye16_d = nc.inline_tensor(
        np.eye(P, dtype=np.float32).astype(ml_dtypes.bfloat16), name="eye16_c")
    tril_d = nc.inline_tensor(
        np.triu(np.ones((P, P), dtype=np.float32)).astype(ml_dtypes.bfloat16),
        name="tril_c")  # keep j <= q
    e65_np = np.zeros((D + 1, 1), dtype=np.float32)
    e65_np[D, 0] = 1.0
    e65_d = nc.inline_tensor(e65_np.astype(ml_dtypes.bfloat16), name="e65_c")

    with tile.TileContext(nc) as tc:
        with (
            tc.tile_pool(name="singles", bufs=1) as singles,
            tc.tile_pool(name="mem", bufs=5) as memp,
            tc.tile_pool(name="mvp", bufs=4) as mvp,
            tc.tile_pool(name="prods", bufs=4) as prods,
            tc.tile_pool(name="small", bufs=10) as small,
            tc.tile_pool(name="pt", bufs=3) as ptp,
            tc.tile_pool(name="stg", bufs=4) as stgp,
            tc.tile_pool(name="outp", bufs=3) as outp,
            tc.tile_pool(name="ppt", bufs=2, space="PSUM") as ppt,
            tc.tile_pool(name="pp512", bufs=2, space="PSUM") as pp512,
            tc.tile_pool(name="ppo", bufs=2, space="PSUM") as ppo,
        ):
            # ---- constants / weights ------------------------------------
            eye16 = singles.tile([P, P], bf16, tag="eye16")
            nc.sync.dma_start(eye16, eye16_d[:, :])
            tril_sb = singles.tile([P, P], bf16, tag="tril")
            nc.sync.dma_start(tril_sb, tril_d[:, :])
            e65 = singles.tile([D + 1, 1], bf16, tag="e65")
            nc.sync.dma_start(e65, e65_d[:, :])
            sc_sb = singles.tile([P, 4], f32, tag="scales")
            nc.sync.dma_start(sc_sb, scales[:, :])
            w_st = singles.tile([P, NCO, 4 * D], f32, tag="wqkv_st")
            nc.sync.dma_start(w_st, wqkv[:, :, :].rearrange("co p c -> p co c"))
            w_sb = singles.tile([P, NCO, 4 * D], f32r, tag="wqkv")
            nc.scalar.copy(out=w_sb, in_=w_st)
            wo_sb = singles.tile([D, 2, DIM], bf16, tag="wo")
            nc.sync.dma_start(wo_sb, wo[:, :, :].rearrange("h d c -> d h c"))

            # ---- xT load (host pre-transposed), 4 n-chunks, round to f32r
            xT = singles.tile([P, NCO, N], f32r, tag="xT")
            for ch in range(4):
                nsl = slice(ch * 512, (ch + 1) * 512)
                x_t = memp.tile([P, NCO, 512], f32, tag="x_in")
                nc.sync.dma_start(x_t, xbT[:, :, nsl].rearrange(
                    "co p n -> p co n"))
                nc.scalar.copy(out=xT[:, :, nsl], in_=x_t)

            # ---- fused q/kv natural projections + sumsq -----------------
            # qkv_sb[:, g, 0:128]  = q for heads p=0,1 (64 cols each)
            # qkv_sb[:, g, 128:192] = k ; 192:256 = v
            qkv_sb = singles.tile([P, NB, 4 * D], f32, tag="qkv")
            # ss cols: 0:16 q0, 16:32 q1, 32:48 k
            ss_all = singles.tile([P, 3 * NB], f32, tag="ss")
            junk = singles.tile([P, D], f32, tag="junk")
            for g in range(NB):
                qsl = slice(g * P, (g + 1) * P)
                qkv_full = pp512.tile([P, 512], f32, tag="st", name="qkv_ps")
                qkv_ps = qkv_full[:, 0:4 * D]
                for co in range(NCO):
                    nc.tensor.matmul(qkv_ps, xT[:, co, qsl], w_sb[:, co, :],
                                     start=(co == 0), stop=(co == NCO - 1))
                nc.scalar.copy(out=qkv_sb[:, g, :], in_=qkv_ps)
                nc.scalar.activation(out=junk, in_=qkv_ps[:, 0:D],
                                     func=ACTF.Square,
                                     accum_out=ss_all[:, g:g + 1])
                nc.scalar.activation(out=junk, in_=qkv_ps[:, D:2 * D],
                                     func=ACTF.Square,
                                     accum_out=ss_all[:, NB + g:NB + g + 1])
                nc.scalar.activation(out=junk, in_=qkv_ps[:, 2 * D:3 * D],
                                     func=ACTF.Square,
                                     accum_out=ss_all[:, 2 * NB + g:2 * NB + g + 1])

            nrm = singles.tile([P, 3 * NB], f32, tag="nrm")
            nc.scalar.sqrt(nrm, ss_all)
            rn = singles.tile([P, 3 * NB], f32, tag="rn")
            nc.vector.reciprocal(rn, nrm)

            # ---- normalized q-hat (bf16), k-hat, v_aug ------------------
            qh = singles.tile([P, 2 * NB, D], bf16, tag="qh")
            kh = singles.tile([P, NB, D], bf16, tag="kh")
            v_aug = singles.tile([P, NB, D + 1], bf16, tag="vaug")
            nc.gpsimd.memset(v_aug[:, :, D:D + 1], 1.0)
            for g in range(NB):
                for p in range(2):
                    nc.gpsimd.tensor_scalar_mul(
                        qh[:, p * NB + g, :], qkv_sb[:, g, p * D:(p + 1) * D],
                        rn[:, p * NB + g:p * NB + g + 1])
                nc.gpsimd.tensor_scalar_mul(
                    kh[:, g, :], qkv_sb[:, g, 2 * D:3 * D],
                    rn[:, 2 * NB + g:2 * NB + g + 1])
                nc.gpsimd.tensor_copy(out=v_aug[:, g, 0:D],
                                      in_=qkv_sb[:, g, 3 * D:4 * D])

            # ---- qT / kT via PE transposes (bf16), batches of 4 ---------
            qT = singles.tile([D, 2 * NB, P], bf16, tag="qT")
            for idxc in range(0, 2 * NB, 4):
                t_ps = ppt.tile([D, 4, P], bf16, tag="tps")
                for i4 in range(4):
                    nc.tensor.transpose(t_ps[:, i4, :], qh[:, idxc + i4, :],
                                        eye16)
                nc.scalar.copy(out=qT[:, idxc:idxc + 4, :], in_=t_ps)
            kT = singles.tile([D, NB, P], bf16, tag="kT")
            for jtc in range(0, NB, 4):
                t_ps = ppt.tile([D, 4, P], bf16, tag="tps")
                for j4 in range(4):
                    nc.tensor.transpose(t_ps[:, j4, :], kh[:, jtc + j4, :],
                                        eye16)
                nc.scalar.copy(out=kT[:, jtc:jtc + 4, :], in_=t_ps)

            # ---- per-head attention -------------------------------------
            # hoT65[p]: rows 0:64 = un-divided head output (d-space),
            # row 64 = softmax denominator; per g-block of 128 queries.
            hoT65 = [singles.tile([D + 1, NB, P], bf16, tag=f"hoT{p}",
                                  name=f"hoT{p}") for p in range(2)]
            r_all = [singles.tile([P, NB], f32, tag=f"r{p}", name=f"rr{p}")
                     for p in range(2)]
            # block-diagonal stage, zeros written once; 4 slots (one per
            # g-block of the active qc-chunk)
            stage4 = singles.tile([P, 4, 4, KNN], bf16, tag="st4")
            nc.gpsimd.memset(stage4, 0.0)

            def scores(p, g):
                """DVE mem-score chain for block (p, g) -> p_mem bf16 +
                block-diag stage4 slot."""
                idx = p * NB + g
                mk_t = memp.tile([P, KNN, D], bf16, tag="mk")
                nc.sync.dma_start(mk_t, mk[p, g])
                prod = prods.tile([P, KNN, D], bf16, tag="prod")
                nc.vector.tensor_mul(
                    prod, mk_t, qh[:, idx, None, :].to_broadcast((P, KNN, D)))
                s_mem = small.tile([P, KNN], f32, tag="smem")
                if SCORE_TREE == 0:
                    nc.vector.reduce_sum(s_mem, prod, axis=AX.X)
                else:
                    w = D
                    for _ in range(SCORE_TREE):
                        h = w // 2
                        nc.vector.tensor_add(prod[:, :, 0:h], prod[:, :, 0:h],
                                             prod[:, :, h:w])
                        w = h
                    if w > 1:
                        nc.vector.reduce_sum(s_mem, prod[:, :, 0:w], axis=AX.X)
                    else:
                        nc.vector.tensor_copy(out=s_mem, in_=prod[:, :, 0])
                if use_mbias:
                    mb_t = small.tile([P, KNN], f32, tag="mbias")
                    nc.sync.dma_start(mb_t, mbias[p, g])
                    nc.vector.tensor_add(s_mem, s_mem, mb_t)
                p_mem = small.tile([P, KNN], bf16, tag="pmem")
                nc.scalar.activation(out=p_mem, in_=s_mem, func=ACTF.Exp,
                                     bias=sc_sb[:, 2 + p:3 + p],
                                     scale=sc_sb[:, p:p + 1])
                # 32x32 band-local transpose -> block-diagonal content
                stD = stgp.tile([P, KNN], bf16, tag="stD")
                nc.vector.transpose(stD, p_mem)
                slot = stage4[:, g % 4, :, :]
                for ql in range(4):
                    nc.vector.tensor_copy(
                        out=slot[32 * ql:32 * (ql + 1), ql, :],
                        in_=stD[32 * ql:32 * (ql + 1), :])
                return slot

            for p in range(2):
                sc_ap = sc_sb[:, p:p + 1]
                nb_ap = sc_sb[:, 2 + p:3 + p]
                for qc in range(4):
                    g_lo4 = 4 * qc
                    # mem scores first: DVE races ahead of the PE local chunk
                    slots = [scores(p, g_lo4 + gi) for gi in range(4)]
                    psum_oT = ppo.tile([D + 1, 4, P], f32, tag="po")
                    # local attention, [65, q] orientation
                    for jt in range(4 * qc + 4):
                        g_lo = max(jt, g_lo4)
                        ng = g_lo4 + 4 - g_lo
                        i_lo = p * NB + g_lo
                        st_ps = pp512.tile([P, 512], f32, tag="st")
                        nc.tensor.matmul(
                            st_ps[:, :ng * P], kT[:, jt, :],
                            qT[:, i_lo:i_lo + ng, :], start=True, stop=True)
                        p_t = ptp.tile([P, 4, P], bf16, tag="pt")
                        nc.scalar.activation(
                            out=p_t[:, :ng, :],
                            in_=st_ps[:, :ng * P].rearrange(
                                "p (g q) -> p g q", q=P),
                            func=ACTF.Exp, bias=nb_ap, scale=sc_ap)
                        if g_lo <= jt < g_lo4 + 4:
                            di = jt - g_lo
                            nc.vector.tensor_mul(p_t[:, di, :], p_t[:, di, :],
                                                 tril_sb)
                        nc.tensor.matmul(
                            psum_oT[:, g_lo - g_lo4:, :], v_aug[:, jt, :],
                            p_t[:, :ng, :], start=(jt == 0), stop=False)
                    # mem values: accumulate straight into psum_oT
                    for gi in range(4):
                        g = g_lo4 + gi
                        mv_t = mvp.tile([P, KNN, D + 1], bf16, tag="mv")
                        nc.sync.dma_start(mv_t, mv[p, g])
                        ovw = psum_oT[:, gi, :].rearrange(
                            "p (ql gf) -> p gf ql", gf=KNN)
                        for g4 in range(KNN):
                            nc.tensor.matmul(
                                ovw[:, g4, :], mv_t[:, g4, :],
                                slots[gi][:, :, g4], start=False,
                                stop=(gi == 3 and g4 == KNN - 1))
                    # un-divided head rows + denominator row -> SBUF
                    nc.scalar.copy(out=hoT65[p][:, g_lo4:g_lo4 + 4, :],
                                   in_=psum_oT)
                    # denominator row -> q-space: lhsT.T @ e65 per g
                    den_full = pp512.tile([P, 512], f32, tag="st", name="den")
                    den_t = den_full[:, 0:4]
                    for gi in range(4):
                        g = g_lo4 + gi
                        nc.tensor.matmul(den_t[:, gi:gi + 1],
                                         hoT65[p][:, g, :], e65,
                                         start=True, stop=True)
                    nc.vector.reciprocal(r_all[p][:, g_lo4:g_lo4 + 4], den_t)

            # ---- per-head output projection + divide + combine ----------
            for g in range(NB):
                pf0 = pp512.tile([P, DIM], f32, tag="st", name="pf0")
                nc.tensor.matmul(pf0, hoT65[0][0:D, g, :], wo_sb[:, 0, :],
                                 start=True, stop=True)
                pf1 = pp512.tile([P, DIM], f32, tag="st", name="pf1")
                nc.tensor.matmul(pf1, hoT65[1][0:D, g, :], wo_sb[:, 1, :],
                                 start=True, stop=True)
                of_s = outp.tile([P, DIM], f32, tag="ofs")
                nc.scalar.mul(of_s, pf0, r_all[0][:, g:g + 1])
                tmp1 = outp.tile([P, DIM], f32, tag="tmp1")
                nc.scalar.mul(tmp1, pf1, r_all[1][:, g:g + 1])
                nc.gpsimd.tensor_add(of_s, of_s, tmp1)
                nc.sync.dma_start(out[g * P:(g + 1) * P, :], of_s)

    nc.compile()
    return nc


def _prep_mv(mv_slice):
    """[2,2048,32,64] -> [2,16,128,32,65] bf16: partition (ql k) stacks the 4
    stride-32 queries of each group; col 64 = 1.0 (softmax-denominator)."""
    import ml_dtypes
    r = mv_slice.reshape(2, NB, 4, KNN, KNN, D).transpose(0, 1, 2, 4, 3, 5)
    out = np.empty((2, NB, P, KNN, D + 1), dtype=ml_dtypes.bfloat16)
    out[..., :D] = r.reshape(2, NB, P, KNN, D).astype(ml_dtypes.bfloat16)
    out[..., D] = 1.0
    return out


def _prepare_in_maps(x, w_q, w_kv, w_out, scale_param, mem_k, mem_v, mem_mask,
                     use_mbias):
    import ml_dtypes
    f = np.float32
    bf = ml_dtypes.bfloat16
    scales8 = np.exp(scale_param.reshape(HEADS).astype(f))
    in_maps = []
    for c in range(NCORES):
        b = c // 4
        h0 = 2 * (c % 4)
        sc = np.empty((P, 4), dtype=f)
        sc[:, 0] = scales8[h0]
        sc[:, 1] = scales8[h0 + 1]
        sc[:, 2] = -scales8[h0]
        sc[:, 3] = -scales8[h0 + 1]
        wcat = np.concatenate(
            [w_q[:, h0 * D:(h0 + 2) * D], w_kv], axis=1).astype(f)
        m = {
            "xbT": np.ascontiguousarray(
                x[b].T.reshape(NCO, P, N), dtype=f),
            "wqkv": np.ascontiguousarray(wcat.reshape(NCO, P, 4 * D)),
            "wo": np.ascontiguousarray(
                w_out[h0 * D:(h0 + 2) * D, :].reshape(2, D, DIM).astype(bf)),
            "scales": sc,
            "mk": np.ascontiguousarray(
                mem_k[b, h0:h0 + 2].reshape(2, NB, P, KNN, D).astype(bf)),
            "mv": _prep_mv(mem_v[b, h0:h0 + 2]),
        }
        if use_mbias:
            mb = np.where(mem_mask[b, h0:h0 + 2], f(0), f(-1e30)).astype(f)
            m["mbias"] = np.ascontiguousarray(mb.reshape(2, NB, P, KNN))
        in_maps.append(m)
    return in_maps


def _run(x, w_q, w_kv, w_out, scale_param, mem_k, mem_v, mem_mask, trace=False):
    from concourse.bass_utils import run_bass_kernel_spmd

    use_mbias = not bool(np.all(mem_mask))
    nc = _build(use_mbias)
    in_maps = _prepare_in_maps(x, w_q, w_kv, w_out, scale_param,
                               mem_k, mem_v, mem_mask, use_mbias)
    res = run_bass_kernel_spmd(nc, in_maps, core_ids=list(range(NCORES)),
                               trace=trace)
    out = np.zeros((B, N, DIM), dtype=np.float32)
    for c in range(NCORES):
        out[c // 4] += res.results[c]["out"]
    return out, res


def kernel(x, w_q, w_kv, w_out, scale_param, mem_k, mem_v, mem_mask):
    trace = bool(int(os.environ.get("BASS_KERNEL_TRACE", "0")))
    out, _ = _run(x, w_q, w_kv, w_out, scale_param, mem_k, mem_v, mem_mask,
                  trace=trace)
    return out
